# revision 22
# baseline (speedup 1.0000x reference)
"""Multi-head attention forward, distributed over 8 TRN2 NeuronCores.

Problem: x[2,2048,1024] -> QKV proj (16 heads x 64) -> softmax attention
-> output proj + bias -> [2,2048,1024], f32 I/O, bf16 tensor-engine compute.

Sharding (v4): rows = flattened (batch, seq) = 4096 rows; core c owns rows
[c*512, (c+1)*512) -- cores 0-3 hold batch 0, cores 4-7 batch 1. Each core
projects Q/K^T/V for its own 512 rows, then K^T AND V are all-gathered
(bf16, 4 pipelined chunks of 128 local keys each) within the 4-core batch
group. Attention runs in two halves of 8 key tiles: within a half each
head pair's output accumulates across its 8 tiles in one PSUM group, and
the halves combine with a single DVE copy+add per pair.

v4 design notes (vs the v3 baseline, 325847 ns measured):
- V is gathered along with K^T, not recomputed per-core from replicated x:
  saves 192 PE matmuls (~41us of the 214us v3 PE-busy) and the whole 4MB
  xg input + its SBUF footprint. AllGather per chunk is 512KB in / 2MB out
  per 4-rank group (~10-15us each on the measured TRN2 collective tables),
  4 chunks pipelined behind the projections.
- Gathered chunks land via 2KB-row DMAs into an SBUF bounce, then DVE
  strided copies scatter into kt (K^T, all keys) and v_aug (V + ones col).
  v3 DMA'd the K^T unpack directly with 256B descriptors (the v2 lesson
  says those run ~6GB/s); the bounce+DVE path avoids that entirely.
- Two-half attention: head-pair-outer over all 16 tiles would block every
  engine FIFO at each AllGather chunk boundary (head-of-line); fully
  chunk-outer (v3) needs SBUF accumulation on DVE after every chunk
  (~39us of DVE). Two halves cost one copy+add per pair (~19us) and keep
  the FIFOs streaming.
- The ones column of v_aug is memset on DVE, not GpSimd, so nothing on the
  Pool queue sits behind the serialized AllGather instructions.
- kvb/pT pools open before the phase-1 weight pools so their SBUF does not
  alias the freed weight region (that aliasing would gate the first unpack
  on the last Wq read, stalling attention by ~15us).
- DMA issue order prioritizes xT+Wk+Wv so the K/V projections (which feed
  the AllGather chain) start as early as possible; Wq loads after the
  chunk-0 staging, Wo/bo during the gather flight.
- Scores contract over K=64 via PE row-tiling: head 2p in partitions 0-63,
  head 2p+1 in 64-127 of the kt/qT pair tiles; the two matmuls run
  concurrently in different PE row-groups.
- V is augmented with a ones column so the attention matmul itself yields
  the softmax denominator in row 64 (exp has no max subtraction; scores
  are ~N(0,1) after the 1/sqrt(64) scale folded into the ACT scale).
- The ScalarE exp stream (128 activations of [128,1024] @ ~1us each) is
  the pacing resource for the attention phase; per-item PE work (2 score
  MMs + 2 A.V MMs, N=512 each, ~650ns with row-group concurrency) fits
  inside it. Everything bf16: fp8 anywhere on the value/score path would
  multiply the ~5e-3 bf16 error past the 2e-2 gate.
- Local MultiCoreSim puts this build at ~215us with realistic collective
  costs (the sim's own collective model charges 15us + bytes/40GBps and
  reports ~420us; see sim_bench.py --mock for the calibrated variant).
"""

import ml_dtypes
import numpy as np

import concourse.bass as bass
import concourse.mybir as mybir
import concourse.tile as tile
from concourse import bacc
from concourse.bass_utils import run_bass_kernel_spmd

BF = mybir.dt.bfloat16
F32 = mybir.dt.float32
P = 128

N_CORES = 8
GROUP = 4   # cores per batch group (one AllGather group)
NCH = 4     # AllGather chunks (pipelined)


class Cfg:
    def __init__(self, rpc, d, n_heads, head_dim):
        self.RPC = rpc            # query rows per core
        self.D = d                # model dim
        self.H = n_heads
        self.HD = head_dim
        assert n_heads * head_dim == d
        self.NT_D = d // P        # dim tiles (= head pairs)
        self.NT_R = rpc // P      # row tiles
        self.KEYS = rpc * GROUP   # keys per batch group
        self.NT_K = self.KEYS // P
        self.KPC = rpc // NCH     # local keys per chunk
        assert self.KPC == P      # one key tile per (rank, chunk)


FULL = Cfg(rpc=512, d=1024, n_heads=16, head_dim=64)


def _body(tc, nc, cfg, xT_in, wq_in, wk_in, wv_in, wo_in, bo_in, out_ext,
          mock_ag=False):
    c = cfg
    AF = mybir.ActivationFunctionType
    rg = [list(range(GROUP)), list(range(GROUP, 2 * GROUP))]
    HD1 = c.HD + 1
    from contextlib import ExitStack

    stack = ExitStack()
    dram = stack.enter_context(tc.tile_pool(name="dram", bufs=1, space="DRAM"))
    const = stack.enter_context(tc.tile_pool(name="const", bufs=1))
    persist = stack.enter_context(tc.tile_pool(name="persist", bufs=1))

    # per chunk: rows 0-127 = K^T chunk, rows 128-255 = V chunk
    kv_in = [dram.tile([2 * P, c.D], BF, name=f"kv_in{h}") for h in range(NCH)]
    kv_g = [
        dram.tile([GROUP * 2 * P, c.D], BF, name=f"kv_g{h}") for h in range(NCH)
    ]
    if mock_ag:
        # Timing-only mode for the local simulator, whose collective cost
        # model charges an unrealistic 15us + bytes/40GB/s: gather a 1KB
        # token instead (cost ~= the ~10-15us a real 2MB 4-rank AllGather
        # takes per the measured TRN2 collectives table), then write the
        # token into kv_in so the unpack keeps the exact same dependency
        # chain (staging -> AG -> unpack). Unpack reads the core's own
        # staged kv_in (right sizes/costs, wrong data for 3 of 4 ranks).
        ag_in = [dram.tile([2, 64], BF, name=f"agi{h}") for h in range(NCH)]
        ag_out = [dram.tile([8, 64], BF, name=f"ago{h}") for h in range(NCH)]
    ones_row = const.tile([1, P], BF, tag="ones_row", name="ones_row")
    nc.vector.memset(ones_row[:], 1.0)
    # Pre-warm the ACT exp table so the ~2.7us table load is off the
    # attention critical path.
    warm_act = const.tile([1, P], BF, tag="warm_act", name="warm_act")
    nc.scalar.activation(warm_act[:], ones_row[:], AF.Exp)

    def ptiles(shape, dt_, pfx, n, pool=None):
        pool = pool or persist
        return [pool.tile(shape, dt_, tag=f"{pfx}{t}", name=f"{pfx}{t}") for t in range(n)]

    xT = ptiles([P, c.RPC], BF, "xT", c.NT_D)
    qT = ptiles([P, c.RPC], BF, "qT", c.NT_D)
    attT = ptiles([P, c.RPC], BF, "attT", c.NT_D)
    # kt: K^T for all keys, one wide tile; pair m's rows at col block m*KEYS
    kt = persist.tile([P, c.NT_D * c.KEYS], BF, tag="kt", name="kt")
    v_aug = ptiles([P, c.H * HD1], BF, "va", c.NT_K)    # V + ones col, per key tile
    # cross-half attention accumulator (bf16, partitions 0-64 only; bf16
    # rounding of the half-sums adds ~0.4% on the denominator path, well
    # inside the error budget, and halves the SBUF footprint)
    acc = ptiles([HD1, 2 * c.RPC], BF, "acc", c.NT_D)
    # output-projection accumulator: out[rt] summed over head pairs by DVE
    out_acc = ptiles([P, c.D], F32, "oacc", c.NT_R)
    bo_sb = const.tile([1, c.D], BF, tag="bo", name="bo_sb")

    # kvb/pT open before the phase-1 pools so they get SBUF disjoint from
    # the weight region: carving them from the freed phase-1 weight space
    # would gate the first unpack on the last Wq read (the final Q-proj
    # matmul), stalling the attention start by ~15us.
    kvb_pool = stack.enter_context(tc.tile_pool(name="kvb_pool", bufs=3))
    pT_pool = stack.enter_context(tc.tile_pool(name="pT", bufs=4))

    def emit_ag(h):
        if mock_ag:
            nc.gpsimd.collective_compute(
                "AllGather",
                mybir.AluOpType.bypass,
                replica_groups=rg,
                ins=[ag_in[h][:].opt()],
                outs=[ag_out[h][:].opt()],
            )
            # token write: makes the unpack DMAs (which read kv_in[h]) wait
            # for the AllGather, mirroring the real kv_g dependency. Issued
            # on the Pool queue (which the AGs serialize anyway) so it does
            # not head-of-line-block the sync DMA queue. A second token
            # chains into the next AG's input so the scheduler cannot hoist
            # AG h+1 ahead of this token (the real AGs serialize on ncfw).
            nc.gpsimd.dma_start(kv_in[h][0:1, 0:1], ag_out[h][0:1, 0:1])
            if h + 1 < NCH:
                nc.gpsimd.dma_start(ag_in[h + 1][0:1, 0:1], ag_out[h][0:1, 0:1])
            return
        nc.gpsimd.collective_compute(
            "AllGather",
            mybir.AluOpType.bypass,
            replica_groups=rg,
            ins=[kv_in[h][:].opt()],
            outs=[kv_g[h][:].opt()],
        )

    with (
        tc.tile_pool(name="stage", bufs=1) as stage,
        tc.tile_pool(name="wpool", bufs=1) as wpool,
        tc.tile_pool(name="proj_psum", bufs=3, space="PSUM") as proj_psum,
    ):
        wk_sb = ptiles([P, c.D], BF, "wk", c.NT_D, pool=wpool)
        wv_sb = ptiles([P, c.D], BF, "wv", c.NT_D, pool=wpool)
        wq_sb = ptiles([P, c.D], BF, "wq", c.NT_D, pool=wpool)
        # wide K^T staging tile: ktw[p, h*D + m*P + k] = K^T[m*P+p, h*KPC+k]
        ktw = stage.tile([P, NCH * c.D], BF, tag="ktw", name="ktw")
        vstage = stage.tile([P, c.D], BF, tag="vstage", name="vstage", bufs=2)

        # ---- phase 0a: priority loads: xT, Wk, Wv (feed the gather chain)
        for t in range(c.NT_D):
            nc.sync.dma_start(xT[t][:], xT_in[t * P : (t + 1) * P, :])
            nc.sync.dma_start(wk_sb[t][:], wk_in[t * P : (t + 1) * P, :])
        for t in range(c.NT_D):
            nc.sync.dma_start(wv_sb[t][:], wv_in[t * P : (t + 1) * P, :])

        # ---- phase 1a: full-width K^T projection, evac sliced per chunk ----
        for m in range(c.NT_D):
            ps = proj_psum.tile([P, c.RPC], F32, tag="proj", name="kproj_ps")
            for k in range(c.NT_D):
                nc.tensor.matmul(
                    ps[:],
                    wk_sb[k][:, m * P : (m + 1) * P],
                    xT[k][:],
                    start=(k == 0),
                    stop=(k == c.NT_D - 1),
                )
            # one strided copy scatters the 4 chunk slices of this m
            nc.vector.tensor_copy(
                ktw[:].rearrange("p (h mk) -> p h mk", h=NCH)[
                    :, :, m * P : (m + 1) * P
                ],
                ps[:].rearrange("p (h k) -> p h k", h=NCH),
            )

        # ---- phase 1b: per chunk: V proj (own keys) -> stage K+V -> AG ----
        for h in range(NCH):
            for n in range(2):
                ps = proj_psum.tile([P, c.RPC], F32, tag="proj", name="vproj_ps")
                for k in range(c.NT_D):
                    nc.tensor.matmul(
                        ps[:],
                        xT[k][:, h * P : (h + 1) * P],
                        wv_sb[k][:, n * c.RPC : (n + 1) * c.RPC],
                        start=(k == 0),
                        stop=(k == c.NT_D - 1),
                    )
                nc.vector.tensor_copy(
                    vstage[:, n * c.RPC : (n + 1) * c.RPC], ps[:]
                )
            nc.sync.dma_start(kv_in[h][0:P, :], ktw[:, h * c.D : (h + 1) * c.D])
            nc.sync.dma_start(kv_in[h][P : 2 * P, :], vstage[:])
            if mock_ag:
                nc.sync.dma_start(ag_in[h][:], ktw[0:2, 0:64])
            emit_ag(h)
            if h == 0:
                # Wq loads slot in after chunk-0 staging; Q proj itself runs
                # after all AGs are in flight (phase 1c).
                for t in range(c.NT_D):
                    nc.sync.dma_start(wq_sb[t][:], wq_in[t * P : (t + 1) * P, :])

        # ---- phase 1c: Q projection (overlaps AllGather flight) ----
        for m in range(c.NT_D):
            ps = proj_psum.tile([P, c.RPC], F32, tag="proj", name="qproj_ps")
            for k in range(c.NT_D):
                nc.tensor.matmul(
                    ps[:],
                    wq_sb[k][:, m * P : (m + 1) * P],
                    xT[k][:],
                    start=(k == 0),
                    stop=(k == c.NT_D - 1),
                )
            nc.vector.tensor_copy(qT[m][:], ps[:])

    # ---- phase 2: unpack gathered K/V as chunks land ----
    # key-tile order is chunk-major: tile j = h*GROUP + r covers gathered
    # chunk h of rank r; kt cols [m*KEYS + j*P, ...) and v_aug[j] use it.
    with (
        tc.tile_pool(name="wopool", bufs=1) as wopool,
        tc.tile_pool(name="small", bufs=4) as small,
        tc.tile_pool(name="sc_psum", bufs=2, space="PSUM") as sc_psum,
        tc.tile_pool(name="att_psum", bufs=2, space="PSUM") as att_psum,
    ):
        wo_sb = ptiles([P, c.D], BF, "wo", c.NT_D, pool=wopool)
        # Wo/bo loads ride behind the staging DMAs, ahead of the unpacks.
        for t in range(c.NT_D):
            nc.sync.dma_start(wo_sb[t][:], wo_in[t * P : (t + 1) * P, :])
        nc.sync.dma_start(bo_sb[:], bo_in[:, :])

        for h in range(NCH):
            for r in range(GROUP):
                j = h * GROUP + r
                kvb = kvb_pool.tile([P, 2 * c.D], BF, tag="kvb", name="kvb")
                # rows r*256..r*256+255 of kv_g[h]: 2KB-contiguous DMA rows
                unpack_src = (
                    kv_in[h][:].rearrange("(u p) q -> p u q", u=2)
                    if mock_ag
                    else kv_g[h][:].rearrange("(b u p) q -> b p u q", b=GROUP, u=2)[r]
                )
                nc.sync.dma_start(
                    kvb[:].rearrange("p (u q) -> p u q", u=2), unpack_src
                )
                # K^T scatter: kt[p, m*KEYS + j*P + k] = kvb[p, m*P + k]
                nc.vector.tensor_copy(
                    kt[:].rearrange("p (m k) -> p m k", m=c.NT_D)[
                        :, :, j * P : (j + 1) * P
                    ],
                    kvb[:, 0 : c.D].rearrange("p (m k) -> p m k", m=c.NT_D),
                )
                # V scatter into the ones-augmented [head, 65] layout
                nc.vector.tensor_copy(
                    v_aug[j][:].rearrange("p (x e) -> p x e", e=HD1)[
                        :, :, 0 : c.HD
                    ],
                    kvb[:, c.D : 2 * c.D].rearrange("p (x e) -> p x e", e=c.HD),
                )
                nc.vector.memset(
                    v_aug[j][:].rearrange("p (x e) -> p x e", e=HD1)[
                        :, :, c.HD : HD1
                    ],
                    1.0,
                )

        # ---- phase 3: attention in two halves of 8 key tiles each.
        # Within a half, a pair's output accumulates across its 8 key tiles
        # in one PSUM group; halves combine via one DVE copy+add per pair.
        # Chunk-major half order keeps the engine FIFOs streaming as each
        # AllGather chunk lands (head-pair-outer over all 16 tiles would
        # head-of-line-block every queue at each chunk boundary).
        JH = c.NT_K // 2
        for half in range(2):
            j0 = half * JH
            for p in range(c.NT_D):
                he, ho = 2 * p, 2 * p + 1
                att_eo = att_psum.tile(
                    [HD1, 2 * c.RPC], F32, tag="att_eo", name="att_eo"
                )
                for j in range(j0, j0 + JH):
                    col = p * c.KEYS + j * P
                    sc = sc_psum.tile([P, 2 * c.RPC], F32, tag="scores", name="sc_ps")
                    # even/odd heads of the pair run concurrently in PE
                    # row-groups 0 and 2 (K=64 row tiling)
                    nc.tensor.matmul(
                        sc[:, 0 : c.RPC],
                        kt[0 : c.HD, col : col + P],
                        qT[p][0 : c.HD, :],
                        start=True,
                        stop=True,
                    )
                    nc.tensor.matmul(
                        sc[:, c.RPC : 2 * c.RPC],
                        kt[c.HD : P, col : col + P],
                        qT[p][c.HD : P, :],
                        start=True,
                        stop=True,
                    )
                    pT = pT_pool.tile([P, 2 * c.RPC], BF, tag="pT", name="pT")
                    nc.scalar.activation(
                        pT[:], sc[:], AF.Exp, scale=1.0 / float(np.sqrt(c.HD))
                    )
                    nc.tensor.matmul(
                        att_eo[:, 0 : c.RPC],
                        v_aug[j][:, he * HD1 : (he + 1) * HD1],
                        pT[:, 0 : c.RPC],
                        start=(j == j0),
                        stop=(j == j0 + JH - 1),
                    )
                    nc.tensor.matmul(
                        att_eo[:, c.RPC : 2 * c.RPC],
                        v_aug[j][:, ho * HD1 : (ho + 1) * HD1],
                        pT[:, c.RPC : 2 * c.RPC],
                        start=(j == j0),
                        stop=(j == j0 + JH - 1),
                    )
                if half == 0:
                    nc.vector.tensor_copy(acc[p][:], att_eo[:])
                    continue
                nc.vector.tensor_add(acc[p][:], att_eo[:], acc[p][:])
                # normalization: denominators live in row HD of acc.
                # reciprocal_approx_fast is a custom DVE program -- feed it
                # a partition-0-based tile, not a row-64 slice.
                den = small.tile([1, 2 * c.RPC], F32, tag="den", name="den", bufs=2)
                nc.vector.tensor_copy(den[:], acc[p][c.HD : HD1, :])
                rcp = small.tile([1, 2 * c.RPC], F32, tag="rcp", name="rcp", bufs=2)
                nc.vector.reciprocal_approx_fast(rcp[:], den[:])
                rcpb = small.tile([c.HD, 2 * c.RPC], F32, tag="rcpb", name="rcpb", bufs=2)
                nc.gpsimd.partition_broadcast(rcpb[:], rcp[:])
                nc.vector.tensor_mul(
                    attT[p][0 : c.HD, :], acc[p][0 : c.HD, 0 : c.RPC],
                    rcpb[:, 0 : c.RPC],
                )
                nc.vector.tensor_mul(
                    attT[p][c.HD : P, :], acc[p][0 : c.HD, c.RPC : 2 * c.RPC],
                    rcpb[:, c.RPC : 2 * c.RPC],
                )
        # ---- phase 4: output projection + bias (PSUM accumulation over
        # the 8 head pairs; a matmul output cannot cross a PSUM bank, so
        # each 512-col half is its own accumulation group) ----
        for rt in range(c.NT_R):
            for n in range(2):
                po = sc_psum.tile([P, c.RPC], F32, tag="scores", name="out_ps")
                for k in range(c.NT_D):
                    nc.tensor.matmul(
                        po[:],
                        attT[k][:, rt * P : (rt + 1) * P],
                        wo_sb[k][:, n * c.RPC : (n + 1) * c.RPC],
                        start=(k == 0),
                        stop=False,
                    )
                nc.tensor.matmul(
                    po[:],
                    ones_row[:],
                    bo_sb[:, n * c.RPC : (n + 1) * c.RPC],
                    start=False,
                    stop=True,
                )
                nc.vector.tensor_copy(
                    out_acc[rt][:, n * c.RPC : (n + 1) * c.RPC], po[:]
                )
            nc.sync.dma_start(out_ext[rt * P : (rt + 1) * P, :], out_acc[rt][:])

    stack.close()


def build_nc(cfg, mock_ag=False):
    nc = bacc.Bacc(
        "TRN2", target_bir_lowering=False, debug=False, num_devices=N_CORES
    )
    c = cfg
    xT_in = nc.dram_tensor("xT", [c.D, c.RPC], BF, kind="ExternalInput")
    wq_in = nc.dram_tensor("Wq", [c.D, c.D], BF, kind="ExternalInput")
    wk_in = nc.dram_tensor("Wk", [c.D, c.D], BF, kind="ExternalInput")
    wv_in = nc.dram_tensor("Wv", [c.D, c.D], BF, kind="ExternalInput")
    wo_in = nc.dram_tensor("Wo", [c.D, c.D], BF, kind="ExternalInput")
    bo_in = nc.dram_tensor("bo", [1, c.D], BF, kind="ExternalInput")
    out_ext = nc.dram_tensor("out", [c.RPC, c.D], F32, kind="ExternalOutput")

    with tile.TileContext(nc) as tc:
        _body(
            tc, nc, cfg,
            xT_in.ap(), wq_in.ap(), wk_in.ap(), wv_in.ap(),
            wo_in.ap(), bo_in.ap(), out_ext.ap(),
            mock_ag=mock_ag,
        )
    nc.compile()
    return nc


_cached_nc = None


def _bf16(a):
    return np.ascontiguousarray(np.asarray(a, dtype=np.float32)).astype(
        ml_dtypes.bfloat16
    )


def prep_in_maps(c, x, Wq, Wk, Wv, Wo, bo):
    xf = np.ascontiguousarray(np.asarray(x, dtype=np.float32)).reshape(-1, c.D)
    wq, wk, wv, wo = _bf16(Wq), _bf16(Wk), _bf16(Wv), _bf16(Wo)
    bob = _bf16(bo).reshape(1, c.D)
    return [
        {
            "xT": np.ascontiguousarray(
                xf[cid * c.RPC : (cid + 1) * c.RPC].T.astype(ml_dtypes.bfloat16)
            ),
            "Wq": wq, "Wk": wk, "Wv": wv, "Wo": wo, "bo": bob,
        }
        for cid in range(N_CORES)
    ]


def kernel(x, Wq, Wk, Wv, Wo, bo):
    global _cached_nc
    c = FULL
    if _cached_nc is None:
        _cached_nc = build_nc(c)
    nc = _cached_nc

    in_maps = prep_in_maps(c, x, Wq, Wk, Wv, Wo, bo)
    res = run_bass_kernel_spmd(nc, in_maps, list(range(N_CORES)))
    out = np.concatenate([res.results[cid]["out"] for cid in range(N_CORES)], axis=0)
    return out.reshape(np.asarray(x).shape).astype(np.float32)
